# revision 8
# baseline (speedup 1.0000x reference)
"""BrickTube kernel for 8x Trainium2 NeuronCores.

The reference "BrickTube" module applies 80 tiny (2,2,2,2) gate cores to a
[B, 1024] state tensor. Every gate application is linear in x and
INPUT_DIM == BINDIM == OUTPUT_DIM == 1024, so the whole module collapses to

    out = x @ W,   W[i, :] = circuit(e_i)  (1024 x 1024)

W is built exactly on the host in float64 from `cores` (cheap: 80 small
tensordots), then the device runs a batch-sharded dense matmul:
each of the 8 cores computes y_c^T = W^T @ x_c^T for its 4096-row shard of x.

Mixed precision: the (k < 512) x (m < 512) quarter of the contraction runs
as fp8e4m3 DoubleRow matmuls (2x PE rate); everything else runs in fp16.
The fp8 partial accumulates in its own PSUM half-banks (DoubleRow output
must sit at PSUM partition 0, so it cannot share the fp16 banks) and is
stored as a separate scaled fp16 tensor yt8; the host adds yt8/c into the
fp16 partial. Measured end-to-end relative error 1.73e-2 (< 2e-2 gate),
deterministic for the fixed-seed inputs and robust to fp8 subnormal flush.

Device schedule (per core), j-outer over 8 batch chunks of 512 columns:
  per j: DR slices 0-3 (banks ps4/ps5) -> phase A fp16 (m<512, k>=512;
  banks ps0-ps3) -> DR slices 4-7 (ps6/ps7) -> B wave 1 (m-chunks 4,5;
  all k; ps4/ps5) -> B wave 2 (m-chunks 6,7; ps6/ps7). Each bank gets a
  multi-us drain window. j0 runs B(k-outer) -> A -> DR so the first
  matmuls only need the first x/w pieces; j7 drains each bank eagerly so
  the kernel tail is one 128KB DMA. Drains split DVE/ACT; output DMA
  triggers alternate the Sync/Scalar HWDGE rings. ~3us of warmup matmuls
  on zeros bring the PE clock to full rate before real data lands.
"""

import math

import ml_dtypes
import numpy as np

# ---- problem constants (hardcoded per contract) ----
B = 32768
D = 1024
N_CORES = 8
NPC = B // N_CORES  # 4096 batch rows per core

BOND = 2
Q = 10
N_LAYERS = 8
PAIRS1 = [(i, i + 1) for i in range(0, Q, 2)]
PAIRS2 = [(i, (i + 1) % Q) for i in range(1, Q, 2)]
HALF = Q // 2

KF = 512           # fp8 k rows (4 DoubleRow k-tiles of 128)
MF = 512           # fp8 m columns (8 slices of 64)
KC = D // 128      # 8 fp16 contraction chunks (full k range)
JC = NPC // 512    # 8 batch column chunks
MC = D // 128      # 8 output-row chunks
FP8_MAX = 240.0    # TRN fp8e4 == ml_dtypes.float8_e4m3 (max normal 240)
C_TARGET = 49152.0  # fp8 product scale sx*sw; keeps |c*y8| ~1.1e4 << fp16 max


def build_w(cores: np.ndarray) -> np.ndarray:
    """Collapse the 80-gate circuit into W [1024, 1024] (float64),
    with out_row = x_row @ W."""
    c = cores.astype(np.float64)
    s = np.eye(D, dtype=np.float64).reshape((D,) + (BOND,) * Q)
    for layer in range(N_LAYERS):
        base = layer * Q
        for g, (i, j) in enumerate(PAIRS1):
            s = np.tensordot(s, c[base + g], axes=((i + 1, j + 1), (0, 1)))
            s = np.moveaxis(s, (-2, -1), (i + 1, j + 1))
        for g, (i, j) in enumerate(PAIRS2):
            s = np.tensordot(s, c[base + HALF + g], axes=((i + 1, j + 1), (0, 1)))
            s = np.moveaxis(s, (-2, -1), (i + 1, j + 1))
    return s.reshape(D, D)


_NC_CACHE = None


def _build_bass():
    """Device program (identical on all 8 cores):
      inputs:  xt16 [128, JC, KC, 512] f16   (p,j,kc,n) = x[j*512+n, kc*128+p]
               xt8  [128, JC, 4, 512] fp8e4  (p,j,i,n)  = sx*x[j*512+n, i*128+p]
               w16  [128, KC, D] f16         (p,kc,m)   = W[kc*128+p, m]
               w8   [128, 4, MF] fp8e4       (p,i,m)    = sw*W[i*128+p, m]
      outputs: yt16 [128, MC, NPC] f16  fp16 partial: full y for m>=512,
                                        k>=512 part for m<512
               yt8  [64, 8, NPC] f16    c*(fp8 partial), m-col slice s=m//64
    """
    global _NC_CACHE
    if _NC_CACHE is not None:
        return _NC_CACHE

    import concourse.bacc as bacc
    import concourse.mybir as mybir
    import concourse.tile as tile

    F8 = mybir.dt.float8e4
    F16 = mybir.dt.float16
    F32 = mybir.dt.float32
    DR = mybir.MatmulPerfMode.DoubleRow

    nc = bacc.Bacc("TRN2")
    xt16 = nc.dram_tensor("xt16", [128, JC, KC, 512], F16, kind="ExternalInput")
    xt8 = nc.dram_tensor("xt8", [128, JC, 4, 512], F8, kind="ExternalInput")
    w16 = nc.dram_tensor("w16", [128, KC, D], F16, kind="ExternalInput")
    w8 = nc.dram_tensor("w8", [128, 4, MF], F8, kind="ExternalInput")
    yt16 = nc.dram_tensor("yt16", [128, MC, NPC], F16, kind="ExternalOutput")
    yt8 = nc.dram_tensor("yt8", [64, MF // 64, NPC], F16, kind="ExternalOutput")

    with tile.TileContext(nc) as tc:
        with (
            tc.tile_pool(name="xpool", bufs=1) as xpool,
            tc.tile_pool(name="wpool", bufs=1) as wpool,
            tc.tile_pool(name="opool", bufs=2) as opool,
            tc.tile_pool(name="psum", bufs=1, space="PSUM") as ppool,
        ):
            # ---- PE warmup (HAM clock ramp) while the first DMAs land.
            warm = xpool.tile([128, 64], F16, name="warm", tag="warm")
            nc.gpsimd.memset(warm[:], 0)
            wps = ppool.tile([128, 64], F32, name="wps", tag="ps7")
            for _ in range(56):
                nc.tensor.matmul(wps[0:64, :], warm[:], warm[:])

            # ---- input loads.
            # Sync ring: x16 j0 per k-chunk (first rhs ASAP), then x8/x16
            # interleaved in consumption order.
            x16t = []
            x8t = []
            for j in range(JC):
                x16t.append(
                    xpool.tile([128, KC * 512], F16, name=f"x16_{j}", tag=f"x16_{j}")
                )
                x8t.append(
                    xpool.tile([128, 4, 512], F8, name=f"x8_{j}", tag=f"x8_{j}")
                )
            for kc in range(KC):
                nc.sync.dma_start(
                    x16t[0][:, kc * 512 : (kc + 1) * 512], xt16[:, 0, kc]
                )
            nc.sync.dma_start(x8t[0][:], xt8[:, 0])
            nc.sync.dma_start(x8t[1][:], xt8[:, 1])
            for j in range(1, JC):
                nc.sync.dma_start(
                    x16t[j][:].rearrange("p (kc n) -> p kc n", n=512), xt16[:, j]
                )
                if 2 <= j <= 3:
                    nc.sync.dma_start(x8t[j][:], xt8[:, j])
            for j in range(4, JC):
                nc.sync.dma_start(x8t[j][:], xt8[:, j])
            # Scalar ring: w16 (k0 split, B-wave half first), then w8.
            w16t = []
            for kc in range(KC):
                wt = wpool.tile([128, D], F16, name=f"w16_{kc}", tag=f"w16_{kc}")
                if kc == 0:
                    nc.scalar.dma_start(wt[:, 512:], w16[:, 0, 512:])
                    nc.scalar.dma_start(wt[:, :512], w16[:, 0, :512])
                else:
                    nc.scalar.dma_start(wt[:], w16[:, kc])
                w16t.append(wt)
            w8t = wpool.tile([128, 4, MF], F8, name="w8", tag="w8")
            nc.scalar.dma_start(w8t[:], w8[:])

            ndma = [0]

            def out_dma(dst, src):
                eng = nc.sync if ndma[0] % 2 == 0 else nc.scalar
                ndma[0] += 1
                eng.dma_start(dst, src)

            def drain16(ps, m, j):
                """Eagerly drain+store fp16 bank m (j7 tail path)."""
                osb = opool.tile([128, 512], F16, name=f"osl{m}", tag=f"osl{m}")
                if m % 2 == 0:
                    nc.vector.tensor_copy(osb[:], ps[:])
                else:
                    nc.scalar.copy(osb[:], ps[:])
                out_dma(yt16[:, m, j * 512 : (j + 1) * 512], osb[:])

            def dr_slice(psb, s, j):
                """Four DoubleRow matmuls accumulating fp8 m-cols
                [64s, 64s+64) x all 512 fp8 k rows into psb[0:64, :]."""
                for nh in range(2):
                    for t in range(2):
                        nc.tensor.matmul(
                            psb[0:64, nh * 256 : (nh + 1) * 256],
                            w8t[:, 2 * t : 2 * t + 2, s * 64 : (s + 1) * 64],
                            x8t[j][:, 2 * t : 2 * t + 2, nh * 256 : (nh + 1) * 256],
                            start=(t == 0),
                            stop=(t == 1),
                            perf_mode=DR,
                        )

            def dr_pair_drain(psa, psb, sp, j):
                """Drain DR slices 2sp, 2sp+1 -> yt8 (one DMA per pair)."""
                osb = opool.tile([64, 1024], F16, name=f"o8_{sp}", tag=f"o8_{sp}")
                nc.vector.tensor_copy(osb[:, :512], psa[0:64, :])
                nc.scalar.copy(osb[:, 512:], psb[0:64, :])
                out_dma(
                    yt8[:, 2 * sp : 2 * sp + 2, j * 512 : (j + 1) * 512],
                    osb[:].rearrange("p (s n) -> p s n", n=512),
                )

            def pair_drain(psa, psb, mp, j):
                """Drain fp16 banks 2mp, 2mp+1 -> yt16 (one DMA per pair)."""
                osb = opool.tile([128, 1024], F16, name=f"osb{mp}", tag=f"osb{mp}")
                nc.vector.tensor_copy(osb[:, :512], psa[:])
                nc.scalar.copy(osb[:, 512:], psb[:])
                out_dma(
                    yt16[:, 2 * mp : 2 * mp + 2, j * 512 : (j + 1) * 512],
                    osb[:].rearrange("p (m n) -> p m n", n=512),
                )

            def a_mm(ps, mc, kc, j, start, stop):
                nc.tensor.matmul(
                    ps[:],
                    w16t[kc][:, mc * 128 : (mc + 1) * 128],
                    x16t[j][:, kc * 512 : (kc + 1) * 512],
                    start=start,
                    stop=stop,
                )

            # ---- main loop.
            for j in range(JC):
                ps = [
                    ppool.tile([128, 512], F32, name=f"ps{m}", tag=f"ps{m}")
                    for m in range(MC)
                ]
                if j == 0:
                    # B: all-k fp16 for m>=512, k-outer so each new w/x chunk
                    # feeds 4 matmuls (matches DMA arrival rate).
                    for kc in range(KC):
                        for mc in range(4, MC):
                            a_mm(ps[mc], mc, kc, 0, kc == 0, kc == KC - 1)
                    pair_drain(ps[4], ps[5], 2, 0)
                    pair_drain(ps[6], ps[7], 3, 0)
                    # A: k>=512 fp16 for m<512 (B drains retire meanwhile).
                    for kc in range(KC // 2, KC):
                        for mc in range(4):
                            a_mm(ps[mc], mc, kc, 0, kc == KC // 2, kc == KC - 1)
                    # DR slices in the freed banks 4..7.
                    for s in range(4):
                        dr_slice(ps[4 + s], s, 0)
                    dr_pair_drain(ps[4], ps[5], 0, 0)
                    dr_pair_drain(ps[6], ps[7], 1, 0)
                    pair_drain(ps[0], ps[1], 0, 0)
                    pair_drain(ps[2], ps[3], 1, 0)
                    for s in range(4, 8):
                        dr_slice(ps[s], s, 0)
                    dr_pair_drain(ps[4], ps[5], 2, 0)
                    dr_pair_drain(ps[6], ps[7], 3, 0)
                elif j < JC - 1:
                    # DR slices 0-3 in ps4..ps7.
                    for s in range(4):
                        dr_slice(ps[4 + s], s, j)
                    dr_pair_drain(ps[4], ps[5], 0, j)
                    dr_pair_drain(ps[6], ps[7], 1, j)
                    # A (ps0-3) while DR drains retire.
                    for kc in range(KC // 2, KC):
                        for mc in range(4):
                            a_mm(ps[mc], mc, kc, j, kc == KC // 2, kc == KC - 1)
                    # DR slices 4-7, reusing ps4..ps7.
                    for s in range(4, 8):
                        dr_slice(ps[s], s, j)
                    dr_pair_drain(ps[4], ps[5], 2, j)
                    dr_pair_drain(ps[6], ps[7], 3, j)
                    pair_drain(ps[0], ps[1], 0, j)
                    pair_drain(ps[2], ps[3], 1, j)
                    # B wave 1 (m-chunks 4,5; ps4/ps5).
                    for kc in range(KC):
                        for mc in (4, 5):
                            a_mm(ps[mc], mc, kc, j, kc == 0, kc == KC - 1)
                    pair_drain(ps[4], ps[5], 2, j)
                    # B wave 2 (m-chunks 6,7; ps6/ps7).
                    for kc in range(KC):
                        for mc in (6, 7):
                            a_mm(ps[mc], mc, kc, j, kc == 0, kc == KC - 1)
                    pair_drain(ps[6], ps[7], 3, j)
                else:
                    # j7: eager per-bank drains so the tail is tiny.
                    for s in range(4):
                        dr_slice(ps[4 + s], s, j)
                    dr_pair_drain(ps[4], ps[5], 0, j)
                    dr_pair_drain(ps[6], ps[7], 1, j)
                    for mc in range(4):
                        for kc in range(KC // 2, KC):
                            a_mm(ps[mc], mc, kc, j, kc == KC // 2, kc == KC - 1)
                        drain16(ps[mc], mc, j)
                    for s in range(4, 8):
                        dr_slice(ps[s], s, j)
                    dr_pair_drain(ps[4], ps[5], 2, j)
                    dr_pair_drain(ps[6], ps[7], 3, j)
                    for mc in range(4, MC):
                        for kc in range(KC):
                            a_mm(ps[mc], mc, kc, j, kc == 0, kc == KC - 1)
                        drain16(ps[mc], mc, j)

    nc.compile()
    _NC_CACHE = nc
    return nc


def _quantize(x: np.ndarray, W: np.ndarray):
    """fp8 bytes for the (k<KF) x (m<MF) block, fp16 for everything else.
    Balanced fp8 scales keep both operands ~52 absmax (no clipping, far
    from the subnormal threshold)."""
    xm = float(np.abs(x[:, :KF]).max())
    wm = float(np.abs(W[:KF, :MF]).max())
    sx = math.sqrt(C_TARGET * wm / xm)
    sw = C_TARGET / sx
    c = sx * sw
    assert sx * xm < FP8_MAX and sw * wm < FP8_MAX
    x8 = (x[:, :KF].astype(np.float64) * sx).astype(ml_dtypes.float8_e4m3)
    w8q = (W[:KF, :MF] * sw).astype(np.float32).astype(ml_dtypes.float8_e4m3)
    x16 = x.astype(np.float16)
    w16q = W.astype(np.float32).astype(np.float16)
    w8d = np.ascontiguousarray(w8q.reshape(4, 128, MF).transpose(1, 0, 2))
    w16d = np.ascontiguousarray(w16q.reshape(KC, 128, D).transpose(1, 0, 2))
    return x8, x16, w8d, w16d, c


def _run(x: np.ndarray, cores: np.ndarray, trace: bool = False, trace_cores=None):
    from concourse.bass_utils import run_bass_kernel_spmd

    W = build_w(cores)
    x8, x16, w8d, w16d, c = _quantize(x, W)

    in_maps = []
    for ci in range(N_CORES):
        sl = slice(ci * NPC, (ci + 1) * NPC)
        xt8_c = np.ascontiguousarray(
            x8[sl].reshape(JC, 512, 4, 128).transpose(3, 0, 2, 1)
        )
        xt16_c = np.ascontiguousarray(
            x16[sl].reshape(JC, 512, KC, 128).transpose(3, 0, 2, 1)
        )
        in_maps.append({"xt8": xt8_c, "xt16": xt16_c, "w8": w8d, "w16": w16d})

    nc = _build_bass()
    kwargs = {}
    if trace_cores is not None:
        kwargs["trace_cores"] = trace_cores
    res = run_bass_kernel_spmd(
        nc, in_maps, core_ids=list(range(N_CORES)), trace=trace, **kwargs
    )

    inv_c = np.float32(1.0 / c)
    y = np.empty((B, D), dtype=np.float32)
    for ci in range(N_CORES):
        # yt16 [128, MC, NPC] -> [NPC, D]; add fp8 partial for m < MF
        yc = res.results[ci]["yt16"].astype(np.float32).transpose(2, 1, 0).reshape(NPC, D)
        y8c = res.results[ci]["yt8"].astype(np.float32).transpose(2, 1, 0).reshape(NPC, MF)
        yc[:, :MF] += y8c * inv_c
        y[ci * NPC : (ci + 1) * NPC, :] = yc
    return y, res


def kernel(x: np.ndarray, cores: np.ndarray) -> np.ndarray:
    y, _ = _run(x, cores, trace=False)
    return y


# revision 9
# speedup vs baseline: 1.1211x; 1.1211x over previous
"""BrickTube kernel for 8x Trainium2 NeuronCores.

The reference "BrickTube" module applies 80 tiny (2,2,2,2) gate cores to a
[B, 1024] state tensor. Every gate application is linear in x and
INPUT_DIM == BINDIM == OUTPUT_DIM == 1024, so the whole module collapses to

    out = x @ W,   W[i, :] = circuit(e_i)  (1024 x 1024)

W is built exactly on the host in float64 from `cores` (cheap: 80 small
tensordots), then the device runs a batch-sharded dense fp16 matmul with
fp32 PSUM accumulation: each of the 8 cores computes y_c^T = W^T @ x_c^T
for its 4096-row shard of x. (fp8 DoubleRow was evaluated and discarded:
the per-matmul LDWEIGHTS issue rate (~109ns per 128-col stationary swap)
caps the DoubleRow stream at fp16-equivalent throughput for this shape.)

Schedule notes (per core, j-outer over 8 batch chunks of 512 columns):
  - j0..j6: k-outer/m-inner into 8 PSUM banks; drains (DVE/ACT split by m
    parity) and pair output DMAs overlap the next chunk's matmuls. Output
    DMA triggers alternate the Sync/Scalar HWDGE rings.
  - j7: m-outer/k-inner so each bank finishes, drains, and stores while
    the next bank computes; the final bank is drained and stored as two
    64KB halves on both rings, so the kernel tail is minimal.
  - Output is fp16 (rounding adds ~2e-4 relative error; the gate is 2e-2),
    halving output traffic and drain cost vs fp32.
  - ~3us of warmup matmuls on zeros while the first DMAs land bring the
    PE clock (HAM ramp) to full rate before real data arrives. The first
    x piece is a single 128KB k-chunk and the first w piece is a 32KB
    column sliver so the first real matmul can start as early as possible.
"""

import math

import ml_dtypes
import numpy as np

# ---- problem constants (hardcoded per contract) ----
B = 32768
D = 1024
N_CORES = 8
NPC = B // N_CORES  # 4096 batch rows per core

BOND = 2
Q = 10
N_LAYERS = 8
PAIRS1 = [(i, i + 1) for i in range(0, Q, 2)]
PAIRS2 = [(i, (i + 1) % Q) for i in range(1, Q, 2)]
HALF = Q // 2

KC = D // 128      # 8 contraction chunks
JC = NPC // 512    # 8 batch column chunks
MC = D // 128      # 8 output-row chunks


def build_w(cores: np.ndarray) -> np.ndarray:
    """Collapse the 80-gate circuit into W [1024, 1024] (float64),
    with out_row = x_row @ W."""
    c = cores.astype(np.float64)
    s = np.eye(D, dtype=np.float64).reshape((D,) + (BOND,) * Q)
    for layer in range(N_LAYERS):
        base = layer * Q
        for g, (i, j) in enumerate(PAIRS1):
            s = np.tensordot(s, c[base + g], axes=((i + 1, j + 1), (0, 1)))
            s = np.moveaxis(s, (-2, -1), (i + 1, j + 1))
        for g, (i, j) in enumerate(PAIRS2):
            s = np.tensordot(s, c[base + HALF + g], axes=((i + 1, j + 1), (0, 1)))
            s = np.moveaxis(s, (-2, -1), (i + 1, j + 1))
    return s.reshape(D, D)


_NC_CACHE = None


def _build_bass():
    """Device program (identical on all 8 cores):
      inputs:  xt16 [128, JC, KC, 512] f16  (p,j,kc,n) = x[j*512+n, kc*128+p]
               w16  [128, KC, D] f16        (p,kc,m)   = W[kc*128+p, m]
      output:  yt16 [128, MC, NPC] f16      (p,m,n)    = y[n, m*128+p]
    """
    global _NC_CACHE
    if _NC_CACHE is not None:
        return _NC_CACHE

    import concourse.bacc as bacc
    import concourse.mybir as mybir
    import concourse.tile as tile

    F16 = mybir.dt.float16
    F32 = mybir.dt.float32

    nc = bacc.Bacc("TRN2")
    xt16 = nc.dram_tensor("xt16", [128, JC, KC, 512], F16, kind="ExternalInput")
    w16 = nc.dram_tensor("w16", [128, KC, D], F16, kind="ExternalInput")
    yt16 = nc.dram_tensor("yt16", [128, MC, NPC], F16, kind="ExternalOutput")

    with tile.TileContext(nc) as tc:
        with (
            tc.tile_pool(name="xpool", bufs=1) as xpool,
            tc.tile_pool(name="wpool", bufs=1) as wpool,
            tc.tile_pool(name="opool", bufs=2) as opool,
            tc.tile_pool(name="psum", bufs=1, space="PSUM") as ppool,
        ):
            # ---- PE warmup (HAM clock ramp) while the first DMAs land.
            warm = xpool.tile([128, 64], F16, name="warm", tag="warm")
            nc.gpsimd.memset(warm[:], 0)
            wps = ppool.tile([128, 64], F32, name="wps", tag="ps7")
            for _ in range(56):
                nc.tensor.matmul(wps[0:64, :], warm[:], warm[:])

            # ---- input loads.
            # Sync ring: x. j0 per k-chunk (first rhs lands ASAP), j1 in two
            # halves, j2+ whole (8KB contiguous per partition row).
            x16t = []
            for j in range(JC):
                xtile = xpool.tile(
                    [128, KC * 512], F16, name=f"x16_{j}", tag=f"x16_{j}"
                )
                if j == 0:
                    for kc in range(KC):
                        nc.sync.dma_start(
                            xtile[:, kc * 512 : (kc + 1) * 512], xt16[:, 0, kc]
                        )
                elif j == 1:
                    h = KC // 2
                    for p in range(2):
                        nc.sync.dma_start(
                            xtile[:, p * h * 512 : (p + 1) * h * 512].rearrange(
                                "p (kc n) -> p kc n", n=512
                            ),
                            xt16[:, 1, p * h : (p + 1) * h],
                        )
                else:
                    nc.sync.dma_start(
                        xtile[:].rearrange("p (kc n) -> p kc n", n=512), xt16[:, j]
                    )
                x16t.append(xtile)
            # Scalar ring: w. k0 in three pieces (first matmul only needs the
            # leading 128 columns), then k1..k7 whole.
            w16t = []
            for kc in range(KC):
                wt = wpool.tile([128, D], F16, name=f"w16_{kc}", tag=f"w16_{kc}")
                if kc == 0:
                    nc.scalar.dma_start(wt[:, :128], w16[:, 0, :128])
                    nc.scalar.dma_start(wt[:, 128:512], w16[:, 0, 128:512])
                    nc.scalar.dma_start(wt[:, 512:], w16[:, 0, 512:])
                else:
                    nc.scalar.dma_start(wt[:], w16[:, kc])
                w16t.append(wt)

            ndma = [0]

            def out_dma(dst, src):
                eng = nc.sync if ndma[0] % 2 == 0 else nc.scalar
                ndma[0] += 1
                eng.dma_start(dst, src)

            def mm(ps, mc, kc, j, start, stop):
                nc.tensor.matmul(
                    ps[:],
                    w16t[kc][:, mc * 128 : (mc + 1) * 128],
                    x16t[j][:, kc * 512 : (kc + 1) * 512],
                    start=start,
                    stop=stop,
                )

            # ---- main loop.
            for j in range(JC):
                ps = [
                    ppool.tile([128, 512], F32, name=f"ps{m}", tag=f"ps{m}")
                    for m in range(MC)
                ]
                if j < JC - 1:
                    # k-outer/m-inner; all 8 banks accumulate in parallel.
                    for kc in range(KC):
                        for mc in range(MC):
                            mm(ps[mc], mc, kc, j, kc == 0, kc == KC - 1)
                    # pair drains: DVE even banks / ACT odd banks; one DMA
                    # per pair, alternating rings.
                    for mp in range(MC // 2):
                        osb = opool.tile(
                            [128, 1024], F16, name=f"osb{mp}", tag=f"osb{mp}"
                        )
                        nc.vector.tensor_copy(osb[:, :512], ps[2 * mp][:])
                        nc.scalar.copy(osb[:, 512:], ps[2 * mp + 1][:])
                        out_dma(
                            yt16[:, 2 * mp : 2 * mp + 2, j * 512 : (j + 1) * 512],
                            osb[:].rearrange("p (m n) -> p m n", n=512),
                        )
                else:
                    # j7: m-outer/k-inner with eager per-bank drain+store so
                    # the kernel tail is one small DMA, not 8 banks' worth.
                    for mc in range(MC):
                        for kc in range(KC):
                            mm(ps[mc], mc, kc, j, kc == 0, kc == KC - 1)
                        osb = opool.tile(
                            [128, 512], F16, name=f"osl{mc}", tag=f"osl{mc}"
                        )
                        if mc < MC - 1:
                            if mc % 2 == 0:
                                nc.vector.tensor_copy(osb[:], ps[mc][:])
                            else:
                                nc.scalar.copy(osb[:], ps[mc][:])
                            out_dma(yt16[:, mc, j * 512 : (j + 1) * 512], osb[:])
                        else:
                            # final bank: two half drains + two 64KB DMAs on
                            # both rings to minimize the tail.
                            nc.vector.tensor_copy(osb[:, :256], ps[mc][:, :256])
                            nc.scalar.copy(osb[:, 256:], ps[mc][:, 256:])
                            base = j * 512
                            nc.sync.dma_start(
                                yt16[:, mc, base : base + 256], osb[:, :256]
                            )
                            nc.scalar.dma_start(
                                yt16[:, mc, base + 256 : base + 512], osb[:, 256:]
                            )

    nc.compile()
    _NC_CACHE = nc
    return nc


def _run(x: np.ndarray, cores: np.ndarray, trace: bool = False, trace_cores=None):
    from concourse.bass_utils import run_bass_kernel_spmd

    W = build_w(cores)
    x16 = x.astype(np.float16)
    w16d = np.ascontiguousarray(
        W.astype(np.float32).astype(np.float16).reshape(KC, 128, D).transpose(1, 0, 2)
    )

    in_maps = []
    for ci in range(N_CORES):
        sl = slice(ci * NPC, (ci + 1) * NPC)
        xt16_c = np.ascontiguousarray(
            x16[sl].reshape(JC, 512, KC, 128).transpose(3, 0, 2, 1)
        )
        in_maps.append({"xt16": xt16_c, "w16": w16d})

    nc = _build_bass()
    kwargs = {}
    if trace_cores is not None:
        kwargs["trace_cores"] = trace_cores
    res = run_bass_kernel_spmd(
        nc, in_maps, core_ids=list(range(N_CORES)), trace=trace, **kwargs
    )

    y = np.empty((B, D), dtype=np.float32)
    for ci in range(N_CORES):
        # yt16 [128, MC, NPC] -> [NPC, D]
        y[ci * NPC : (ci + 1) * NPC, :] = (
            res.results[ci]["yt16"].astype(np.float32).transpose(2, 1, 0).reshape(NPC, D)
        )
    return y, res


def kernel(x: np.ndarray, cores: np.ndarray) -> np.ndarray:
    y, _ = _run(x, cores, trace=False)
    return y


# revision 12
# speedup vs baseline: 1.1275x; 1.0057x over previous
"""BrickTube kernel for 8x Trainium2 NeuronCores.

The reference "BrickTube" module applies 80 tiny (2,2,2,2) gate cores to a
[B, 1024] state tensor. Every gate application is linear in x and
INPUT_DIM == BINDIM == OUTPUT_DIM == 1024, so the whole module collapses to

    out = x @ W,   W[i, :] = circuit(e_i)  (1024 x 1024)

W is built exactly on the host in float64 from `cores` (cheap: 80 small
tensordots), then the device runs a batch-sharded dense fp16 matmul with
fp32 PSUM accumulation: each of the 8 cores computes y_c^T = W^T @ x_c^T
for its 4096-row shard of x. (fp8 DoubleRow was evaluated and discarded:
the per-matmul LDWEIGHTS issue rate (~109ns per 128-col stationary swap)
caps the DoubleRow stream at fp16-equivalent throughput for this shape.)

Schedule notes (per core, j-outer over 8 batch chunks of 512 columns):
  - j0..j6: k-outer/m-inner into 8 PSUM banks; drains (DVE/ACT split by m
    parity) and pair output DMAs overlap the next chunk's matmuls. Output
    DMA triggers alternate the Sync/Scalar HWDGE rings.
  - j7: m-outer/k-inner so each bank finishes, drains, and stores while
    the next bank computes; the final bank is drained and stored as two
    64KB halves on both rings, so the kernel tail is minimal.
  - Output is fp16 (rounding adds ~2e-4 relative error; the gate is 2e-2),
    halving output traffic and drain cost vs fp32.
  - ~3us of warmup matmuls on zeros while the first DMAs land bring the
    PE clock (HAM ramp) to full rate before real data arrives. The first
    x piece is a single 128KB k-chunk and the first w piece is a 32KB
    column sliver so the first real matmul can start as early as possible.
"""

import math

import ml_dtypes
import numpy as np

# ---- problem constants (hardcoded per contract) ----
B = 32768
D = 1024
N_CORES = 8
NPC = B // N_CORES  # 4096 batch rows per core

BOND = 2
Q = 10
N_LAYERS = 8
PAIRS1 = [(i, i + 1) for i in range(0, Q, 2)]
PAIRS2 = [(i, (i + 1) % Q) for i in range(1, Q, 2)]
HALF = Q // 2

KC = D // 128      # 8 contraction chunks
JC = NPC // 512    # 8 batch column chunks
MC = D // 128      # 8 output-row chunks


def build_w(cores: np.ndarray) -> np.ndarray:
    """Collapse the 80-gate circuit into W [1024, 1024] (float64),
    with out_row = x_row @ W."""
    c = cores.astype(np.float64)
    s = np.eye(D, dtype=np.float64).reshape((D,) + (BOND,) * Q)
    for layer in range(N_LAYERS):
        base = layer * Q
        for g, (i, j) in enumerate(PAIRS1):
            s = np.tensordot(s, c[base + g], axes=((i + 1, j + 1), (0, 1)))
            s = np.moveaxis(s, (-2, -1), (i + 1, j + 1))
        for g, (i, j) in enumerate(PAIRS2):
            s = np.tensordot(s, c[base + HALF + g], axes=((i + 1, j + 1), (0, 1)))
            s = np.moveaxis(s, (-2, -1), (i + 1, j + 1))
    return s.reshape(D, D)


_NC_CACHE = None


def _build_bass():
    """Device program (identical on all 8 cores):
      inputs:  xt16 [128, JC, KC, 512] f16  (p,j,kc,n) = x[j*512+n, kc*128+p]
               w16  [128, KC, D] f16        (p,kc,m)   = W[kc*128+p, m]
      output:  yt16 [128, MC, NPC] f16      (p,m,n)    = y[n, m*128+p]
    """
    global _NC_CACHE
    if _NC_CACHE is not None:
        return _NC_CACHE

    import concourse.bacc as bacc
    import concourse.mybir as mybir
    import concourse.tile as tile

    F16 = mybir.dt.float16
    F32 = mybir.dt.float32

    nc = bacc.Bacc("TRN2")
    xt16 = nc.dram_tensor("xt16", [128, JC, KC, 512], F16, kind="ExternalInput")
    w16 = nc.dram_tensor("w16", [128, KC, D], F16, kind="ExternalInput")
    yt16 = nc.dram_tensor("yt16", [128, MC, NPC], F16, kind="ExternalOutput")

    with tile.TileContext(nc) as tc:
        with (
            tc.tile_pool(name="xpool", bufs=1) as xpool,
            tc.tile_pool(name="wpool", bufs=1) as wpool,
            tc.tile_pool(name="opool", bufs=2) as opool,
            tc.tile_pool(name="psum", bufs=1, space="PSUM") as ppool,
        ):
            # ---- PE warmup (HAM clock ramp) while the first DMAs land.
            warm = xpool.tile([128, 64], F16, name="warm", tag="warm")
            nc.gpsimd.memset(warm[:], 0)
            wps = ppool.tile([128, 64], F32, name="wps", tag="ps7")
            for _ in range(64):
                nc.tensor.matmul(wps[0:64, :], warm[:], warm[:])

            # ---- input loads.
            # Sync ring: x. j0 per k-chunk (first rhs lands ASAP), j1 in two
            # halves, j2+ whole (8KB contiguous per partition row).
            x16t = []
            for j in range(JC):
                xtile = xpool.tile(
                    [128, KC * 512], F16, name=f"x16_{j}", tag=f"x16_{j}"
                )
                if j == 0:
                    # kc0/kc1/kc2 single chunks (first rhs ASAP), kc3-7 as
                    # one DMA: few triggers (queue issue is ~700ns each)
                    # while each piece's semaphore still fires in time.
                    for kc in range(3):
                        nc.sync.dma_start(
                            xtile[:, kc * 512 : (kc + 1) * 512], xt16[:, 0, kc]
                        )
                    nc.sync.dma_start(
                        xtile[:, 3 * 512 :].rearrange("p (kc n) -> p kc n", n=512),
                        xt16[:, 0, 3:],
                    )
                else:
                    nc.sync.dma_start(
                        xtile[:].rearrange("p (kc n) -> p kc n", n=512), xt16[:, j]
                    )
                x16t.append(xtile)
            # Scalar ring: w. k0 in three pieces (first matmul only needs the
            # leading 128 columns), then k1..k7 whole.
            w16t = []
            for kc in range(KC):
                wt = wpool.tile([128, D], F16, name=f"w16_{kc}", tag=f"w16_{kc}")
                if kc == 0:
                    nc.scalar.dma_start(wt[:, :128], w16[:, 0, :128])
                    nc.scalar.dma_start(wt[:, 128:512], w16[:, 0, 128:512])
                    nc.scalar.dma_start(wt[:, 512:], w16[:, 0, 512:])
                else:
                    nc.scalar.dma_start(wt[:], w16[:, kc])
                w16t.append(wt)

            ndma = [0]

            def out_dma(dst, src):
                eng = nc.sync if ndma[0] % 2 == 0 else nc.scalar
                ndma[0] += 1
                eng.dma_start(dst, src)

            def mm(ps, mc, kc, j, start, stop):
                nc.tensor.matmul(
                    ps[:],
                    w16t[kc][:, mc * 128 : (mc + 1) * 128],
                    x16t[j][:, kc * 512 : (kc + 1) * 512],
                    start=start,
                    stop=stop,
                )

            # ---- main loop.
            for j in range(JC):
                ps = [
                    ppool.tile([128, 512], F32, name=f"ps{m}", tag=f"ps{m}")
                    for m in range(MC)
                ]
                if j < JC - 1:
                    # k-outer/m-inner; all 8 banks accumulate in parallel.
                    for kc in range(KC):
                        for mc in range(MC):
                            mm(ps[mc], mc, kc, j, kc == 0, kc == KC - 1)
                    # pair drains: DVE even banks / ACT odd banks; one DMA
                    # per pair, alternating rings.
                    for mp in range(MC // 2):
                        osb = opool.tile(
                            [128, 1024], F16, name=f"osb{mp}", tag=f"osb{mp}"
                        )
                        nc.vector.tensor_copy(osb[:, :512], ps[2 * mp][:])
                        nc.scalar.copy(osb[:, 512:], ps[2 * mp + 1][:])
                        out_dma(
                            yt16[:, 2 * mp : 2 * mp + 2, j * 512 : (j + 1) * 512],
                            osb[:].rearrange("p (m n) -> p m n", n=512),
                        )
                else:
                    # j7: m-outer/k-inner with eager per-bank drain+store so
                    # the kernel tail is one small DMA, not 8 banks' worth.
                    for mc in range(MC):
                        for kc in range(KC):
                            mm(ps[mc], mc, kc, j, kc == 0, kc == KC - 1)
                        osb = opool.tile(
                            [128, 512], F16, name=f"osl{mc}", tag=f"osl{mc}"
                        )
                        if mc < MC - 1:
                            if mc % 2 == 0:
                                nc.vector.tensor_copy(osb[:], ps[mc][:])
                            else:
                                nc.scalar.copy(osb[:], ps[mc][:])
                            out_dma(yt16[:, mc, j * 512 : (j + 1) * 512], osb[:])
                        else:
                            # final bank: both half drains on DVE (keeps the
                            # Scalar queue free to fire its DMA trigger the
                            # moment the copy lands), 64KB DMAs on both rings.
                            nc.vector.tensor_copy(osb[:, :256], ps[mc][:, :256])
                            nc.vector.tensor_copy(osb[:, 256:], ps[mc][:, 256:])
                            base = j * 512
                            nc.scalar.dma_start(
                                yt16[:, mc, base : base + 256], osb[:, :256]
                            )
                            nc.sync.dma_start(
                                yt16[:, mc, base + 256 : base + 512], osb[:, 256:]
                            )

    nc.compile()
    _NC_CACHE = nc
    return nc


def _run(x: np.ndarray, cores: np.ndarray, trace: bool = False, trace_cores=None):
    from concourse.bass_utils import run_bass_kernel_spmd

    W = build_w(cores)
    x16 = x.astype(np.float16)
    w16d = np.ascontiguousarray(
        W.astype(np.float32).astype(np.float16).reshape(KC, 128, D).transpose(1, 0, 2)
    )

    in_maps = []
    for ci in range(N_CORES):
        sl = slice(ci * NPC, (ci + 1) * NPC)
        xt16_c = np.ascontiguousarray(
            x16[sl].reshape(JC, 512, KC, 128).transpose(3, 0, 2, 1)
        )
        in_maps.append({"xt16": xt16_c, "w16": w16d})

    nc = _build_bass()
    kwargs = {}
    if trace_cores is not None:
        kwargs["trace_cores"] = trace_cores
    res = run_bass_kernel_spmd(
        nc, in_maps, core_ids=list(range(N_CORES)), trace=trace, **kwargs
    )

    y = np.empty((B, D), dtype=np.float32)
    for ci in range(N_CORES):
        # yt16 [128, MC, NPC] -> [NPC, D]
        y[ci * NPC : (ci + 1) * NPC, :] = (
            res.results[ci]["yt16"].astype(np.float32).transpose(2, 1, 0).reshape(NPC, D)
        )
    return y, res


def kernel(x: np.ndarray, cores: np.ndarray) -> np.ndarray:
    y, _ = _run(x, cores, trace=False)
    return y


# revision 15
# speedup vs baseline: 1.1566x; 1.0258x over previous
"""BrickTube kernel for 8x Trainium2 NeuronCores.

The reference "BrickTube" module applies 80 tiny (2,2,2,2) gate cores to a
[B, 1024] state tensor. Every gate application is linear in x and
INPUT_DIM == BINDIM == OUTPUT_DIM == 1024, so the whole module collapses to

    out = x @ W,   W[i, :] = circuit(e_i)  (1024 x 1024)

W is built exactly on the host in float64 from `cores` (cheap: 80 small
tensordots), then the device runs a batch-sharded dense fp16 matmul with
fp32 PSUM accumulation: each of the 8 cores computes y_c^T = W^T @ x_c^T
for its 4096-row shard of x. (fp8 DoubleRow was evaluated and discarded:
the per-matmul LDWEIGHTS issue rate (~109ns per 128-col stationary swap)
caps the DoubleRow stream at fp16-equivalent throughput for this shape.)

Schedule notes (per core, j-outer over 8 batch chunks of 512 columns):
  - j0..j6: k-outer/m-inner into 8 PSUM banks; drains (DVE/ACT split by m
    parity) and pair output DMAs overlap the next chunk's matmuls. Output
    DMA triggers alternate the Sync/Scalar HWDGE rings.
  - j7: m-outer/k-inner so each bank finishes, drains, and stores while
    the next bank computes; the final bank is drained and stored as two
    64KB halves on both rings, so the kernel tail is minimal.
  - Output is fp16 (rounding adds ~2e-4 relative error; the gate is 2e-2),
    halving output traffic and drain cost vs fp32.
  - ~3us of warmup matmuls on zeros while the first DMAs land bring the
    PE clock (HAM ramp) to full rate before real data arrives. The first
    x piece is a single 128KB k-chunk and the first w piece is a 32KB
    column sliver so the first real matmul can start as early as possible.
"""

import math

import ml_dtypes
import numpy as np

# ---- problem constants (hardcoded per contract) ----
B = 32768
D = 1024
N_CORES = 8
NPC = B // N_CORES  # 4096 batch rows per core

BOND = 2
Q = 10
N_LAYERS = 8
PAIRS1 = [(i, i + 1) for i in range(0, Q, 2)]
PAIRS2 = [(i, (i + 1) % Q) for i in range(1, Q, 2)]
HALF = Q // 2

KC = D // 128      # 8 contraction chunks
JC = NPC // 512    # 8 batch column chunks
MC = D // 128      # 8 output-row chunks


def build_w(cores: np.ndarray) -> np.ndarray:
    """Collapse the 80-gate circuit into W [1024, 1024] (float64),
    with out_row = x_row @ W."""
    c = cores.astype(np.float64)
    s = np.eye(D, dtype=np.float64).reshape((D,) + (BOND,) * Q)
    for layer in range(N_LAYERS):
        base = layer * Q
        for g, (i, j) in enumerate(PAIRS1):
            s = np.tensordot(s, c[base + g], axes=((i + 1, j + 1), (0, 1)))
            s = np.moveaxis(s, (-2, -1), (i + 1, j + 1))
        for g, (i, j) in enumerate(PAIRS2):
            s = np.tensordot(s, c[base + HALF + g], axes=((i + 1, j + 1), (0, 1)))
            s = np.moveaxis(s, (-2, -1), (i + 1, j + 1))
    return s.reshape(D, D)


_NC_CACHE = None


def _build_bass():
    """Device program (identical on all 8 cores):
      inputs:  xt16 [128, JC, KC, 512] f16  (p,j,kc,n) = x[j*512+n, kc*128+p]
               w16  [128, KC, D] f16        (p,kc,m)   = W[kc*128+p, m]
      output:  yt16 [128, MC, NPC] f16      (p,m,n)    = y[n, m*128+p]
    """
    global _NC_CACHE
    if _NC_CACHE is not None:
        return _NC_CACHE

    import concourse.bacc as bacc
    import concourse.mybir as mybir
    import concourse.tile as tile

    F16 = mybir.dt.float16
    F32 = mybir.dt.float32

    nc = bacc.Bacc("TRN2")
    xt16 = nc.dram_tensor("xt16", [128, JC, KC, 512], F16, kind="ExternalInput")
    w16 = nc.dram_tensor("w16", [128, KC, D], F16, kind="ExternalInput")
    yt16 = nc.dram_tensor("yt16", [128, MC, NPC], F16, kind="ExternalOutput")

    with tile.TileContext(nc) as tc:
        with (
            tc.tile_pool(name="xpool", bufs=1) as xpool,
            tc.tile_pool(name="wpool", bufs=1) as wpool,
            tc.tile_pool(name="opool", bufs=2) as opool,
            tc.tile_pool(name="psum", bufs=1, space="PSUM") as ppool,
        ):
            # ---- PE warmup (HAM clock ramp) while the first DMAs land.
            warm = xpool.tile([128, 64], F16, name="warm", tag="warm")
            nc.gpsimd.memset(warm[:], 0)
            wps = ppool.tile([128, 64], F32, name="wps", tag="ps7")
            for _ in range(56):
                nc.tensor.matmul(wps[0:64, :], warm[:], warm[:])

            # ---- input loads.
            # Sync ring: x. j0 per k-chunk (first rhs lands ASAP), j1 in two
            # halves, j2+ whole (8KB contiguous per partition row).
            x16t = []
            for j in range(JC):
                xtile = xpool.tile(
                    [128, KC * 512], F16, name=f"x16_{j}", tag=f"x16_{j}"
                )
                if j <= 1:
                    # j0 in 4 pieces of 2 k-chunks, j1 in 2 pieces: piece
                    # cadence matches the DMA ring's ramp-up so the matmul
                    # stream never starves once it starts (starting earlier
                    # on finer pieces just stalls the stream and drops the
                    # PE clock out of its ramped state).
                    pieces = 4 if j == 0 else 2
                    kk = KC // pieces
                    for p in range(pieces):
                        nc.sync.dma_start(
                            xtile[
                                :, p * kk * 512 : (p + 1) * kk * 512
                            ].rearrange("p (kc n) -> p kc n", n=512),
                            xt16[:, j, p * kk : (p + 1) * kk],
                        )
                else:
                    nc.sync.dma_start(
                        xtile[:].rearrange("p (kc n) -> p kc n", n=512), xt16[:, j]
                    )
                x16t.append(xtile)
            # Scalar ring: w. k0 in three pieces (first matmul only needs the
            # leading 128 columns), then k1..k7 whole.
            w16t = []
            for kc in range(KC):
                wt = wpool.tile([128, D], F16, name=f"w16_{kc}", tag=f"w16_{kc}")
                if kc == 0:
                    nc.scalar.dma_start(wt[:, :512], w16[:, 0, :512])
                    nc.scalar.dma_start(wt[:, 512:], w16[:, 0, 512:])
                else:
                    nc.scalar.dma_start(wt[:], w16[:, kc])
                w16t.append(wt)

            ndma = [0]

            def out_dma(dst, src):
                eng = nc.sync if ndma[0] % 2 == 0 else nc.scalar
                ndma[0] += 1
                eng.dma_start(dst, src)

            def mm(ps, mc, kc, j, start, stop):
                nc.tensor.matmul(
                    ps[:],
                    w16t[kc][:, mc * 128 : (mc + 1) * 128],
                    x16t[j][:, kc * 512 : (kc + 1) * 512],
                    start=start,
                    stop=stop,
                )

            # ---- main loop.
            for j in range(JC):
                ps = [
                    ppool.tile([128, 512], F32, name=f"ps{m}", tag=f"ps{m}")
                    for m in range(MC)
                ]
                if j < JC - 1:
                    # k-outer/m-inner; all 8 banks accumulate in parallel.
                    for kc in range(KC):
                        for mc in range(MC):
                            mm(ps[mc], mc, kc, j, kc == 0, kc == KC - 1)
                    # pair drains: DVE even banks / ACT odd banks; one DMA
                    # per pair, alternating rings.
                    for mp in range(MC // 2):
                        osb = opool.tile(
                            [128, 1024], F16, name=f"osb{mp}", tag=f"osb{mp}"
                        )
                        nc.vector.tensor_copy(osb[:, :512], ps[2 * mp][:])
                        nc.scalar.copy(osb[:, 512:], ps[2 * mp + 1][:])
                        out_dma(
                            yt16[:, 2 * mp : 2 * mp + 2, j * 512 : (j + 1) * 512],
                            osb[:].rearrange("p (m n) -> p m n", n=512),
                        )
                else:
                    # j7: m-outer/k-inner with eager per-bank drain+store so
                    # the kernel tail is one small DMA, not 8 banks' worth.
                    for mc in range(MC):
                        for kc in range(KC):
                            mm(ps[mc], mc, kc, j, kc == 0, kc == KC - 1)
                        osb = opool.tile(
                            [128, 512], F16, name=f"osl{mc}", tag=f"osl{mc}"
                        )
                        if mc < MC - 1:
                            if mc % 2 == 0:
                                nc.vector.tensor_copy(osb[:], ps[mc][:])
                            else:
                                nc.scalar.copy(osb[:], ps[mc][:])
                            out_dma(yt16[:, mc, j * 512 : (j + 1) * 512], osb[:])
                        else:
                            # final bank: both half drains on DVE (keeps the
                            # Scalar queue free to fire its DMA trigger the
                            # moment the copy lands), 64KB DMAs on both rings.
                            nc.vector.tensor_copy(osb[:, :256], ps[mc][:, :256])
                            nc.vector.tensor_copy(osb[:, 256:], ps[mc][:, 256:])
                            base = j * 512
                            nc.scalar.dma_start(
                                yt16[:, mc, base : base + 256], osb[:, :256]
                            )
                            nc.sync.dma_start(
                                yt16[:, mc, base + 256 : base + 512], osb[:, 256:]
                            )

    nc.compile()
    _NC_CACHE = nc
    return nc


def _run(x: np.ndarray, cores: np.ndarray, trace: bool = False, trace_cores=None):
    from concourse.bass_utils import run_bass_kernel_spmd

    W = build_w(cores)
    x16 = x.astype(np.float16)
    w16d = np.ascontiguousarray(
        W.astype(np.float32).astype(np.float16).reshape(KC, 128, D).transpose(1, 0, 2)
    )

    in_maps = []
    for ci in range(N_CORES):
        sl = slice(ci * NPC, (ci + 1) * NPC)
        xt16_c = np.ascontiguousarray(
            x16[sl].reshape(JC, 512, KC, 128).transpose(3, 0, 2, 1)
        )
        in_maps.append({"xt16": xt16_c, "w16": w16d})

    nc = _build_bass()
    kwargs = {}
    if trace_cores is not None:
        kwargs["trace_cores"] = trace_cores
    res = run_bass_kernel_spmd(
        nc, in_maps, core_ids=list(range(N_CORES)), trace=trace, **kwargs
    )

    y = np.empty((B, D), dtype=np.float32)
    for ci in range(N_CORES):
        # yt16 [128, MC, NPC] -> [NPC, D]
        y[ci * NPC : (ci + 1) * NPC, :] = (
            res.results[ci]["yt16"].astype(np.float32).transpose(2, 1, 0).reshape(NPC, D)
        )
    return y, res


def kernel(x: np.ndarray, cores: np.ndarray) -> np.ndarray:
    y, _ = _run(x, cores, trace=False)
    return y


# revision 21
# speedup vs baseline: 1.1580x; 1.0012x over previous
"""BrickTube kernel for 8x Trainium2 NeuronCores.

The reference "BrickTube" module applies 80 tiny (2,2,2,2) gate cores to a
[B, 1024] state tensor. Every gate application is linear in x and
INPUT_DIM == BINDIM == OUTPUT_DIM == 1024, so the whole module collapses to

    out = x @ W,   W[i, :] = circuit(e_i)  (1024 x 1024)

W is built exactly on the host in float64 from `cores` (cheap: 80 small
tensordots), then the device runs a batch-sharded dense fp16 matmul with
fp32 PSUM accumulation: each of the 8 cores computes y_c^T = W^T @ x_c^T
for its 4096-row shard of x. (fp8 DoubleRow was evaluated and discarded:
the per-matmul LDWEIGHTS issue rate (~109ns per 128-col stationary swap)
caps the DoubleRow stream at fp16-equivalent throughput for this shape.)

Schedule notes (per core, j-outer over 8 batch chunks of 512 columns):
  - j0..j6: k-outer/m-inner into 8 PSUM banks; drains (DVE/ACT split by m
    parity) and pair output DMAs overlap the next chunk's matmuls. Output
    DMA triggers alternate the Sync/Scalar HWDGE rings.
  - j7: m-outer/k-inner so each bank finishes, drains, and stores while
    the next bank computes; the final bank is drained and stored as two
    64KB halves on both rings, so the kernel tail is minimal.
  - Output is fp16 (rounding adds ~2e-4 relative error; the gate is 2e-2),
    halving output traffic and drain cost vs fp32.
  - ~3us of warmup matmuls on zeros while the first DMAs land bring the
    PE clock (HAM ramp) to full rate before real data arrives. The first
    x piece is a single 128KB k-chunk and the first w piece is a 32KB
    column sliver so the first real matmul can start as early as possible.
"""

import math

import ml_dtypes
import numpy as np

# ---- problem constants (hardcoded per contract) ----
B = 32768
D = 1024
N_CORES = 8
NPC = B // N_CORES  # 4096 batch rows per core

BOND = 2
Q = 10
N_LAYERS = 8
PAIRS1 = [(i, i + 1) for i in range(0, Q, 2)]
PAIRS2 = [(i, (i + 1) % Q) for i in range(1, Q, 2)]
HALF = Q // 2

KC = D // 128      # 8 contraction chunks
JC = NPC // 512    # 8 batch column chunks
MC = D // 128      # 8 output-row chunks


def build_w(cores: np.ndarray) -> np.ndarray:
    """Collapse the 80-gate circuit into W [1024, 1024] (float64),
    with out_row = x_row @ W."""
    c = cores.astype(np.float64)
    s = np.eye(D, dtype=np.float64).reshape((D,) + (BOND,) * Q)
    for layer in range(N_LAYERS):
        base = layer * Q
        for g, (i, j) in enumerate(PAIRS1):
            s = np.tensordot(s, c[base + g], axes=((i + 1, j + 1), (0, 1)))
            s = np.moveaxis(s, (-2, -1), (i + 1, j + 1))
        for g, (i, j) in enumerate(PAIRS2):
            s = np.tensordot(s, c[base + HALF + g], axes=((i + 1, j + 1), (0, 1)))
            s = np.moveaxis(s, (-2, -1), (i + 1, j + 1))
    return s.reshape(D, D)


_NC_CACHE = None


def _build_bass():
    """Device program (identical on all 8 cores):
      inputs:  xt16 [128, JC, KC, 512] f16  (p,j,kc,n) = x[j*512+n, kc*128+p]
               w16  [128, KC, D] f16        (p,kc,m)   = W[kc*128+p, m]
      output:  yt16 [128, MC, NPC] f16      (p,m,n)    = y[n, m*128+p]
    """
    global _NC_CACHE
    if _NC_CACHE is not None:
        return _NC_CACHE

    import concourse.bacc as bacc
    import concourse.mybir as mybir
    import concourse.tile as tile

    F16 = mybir.dt.float16
    F32 = mybir.dt.float32

    nc = bacc.Bacc("TRN2")
    xt16 = nc.dram_tensor("xt16", [D, NPC], F16, kind="ExternalInput")
    w16 = nc.dram_tensor("w16", [128, KC, D], F16, kind="ExternalInput")
    yt16 = nc.dram_tensor("yt16", [128, MC, NPC], F16, kind="ExternalOutput")

    with tile.TileContext(nc) as tc:
        with (
            tc.tile_pool(name="xpool", bufs=1) as xpool,
            tc.tile_pool(name="wpool", bufs=1) as wpool,
            tc.tile_pool(name="opool", bufs=2) as opool,
            tc.tile_pool(name="psum", bufs=1, space="PSUM") as ppool,
        ):
            # ---- PE warmup (HAM clock ramp) while the first DMAs land.
            warm = xpool.tile([128, 64], F16, name="warm", tag="warm")
            nc.gpsimd.memset(warm[:], 0)
            wps = ppool.tile([128, 64], F32, name="wps", tag="ps7")
            for _ in range(56):
                nc.tensor.matmul(wps[0:64, :], warm[:], warm[:])

            # ---- input loads.
            # Sync ring: x. j0 per k-chunk (first rhs lands ASAP), j1 in two
            # halves, j2+ whole (8KB contiguous per partition row).
            x16t = []
            for j in range(JC):
                xtile = xpool.tile(
                    [128, KC * 512], F16, name=f"x16_{j}", tag=f"x16_{j}"
                )
                # j0 in 4 pieces of 2 k-chunks, j1 in 2 pieces, rest whole:
                # this piece cadence matches the DMA ring's ramp-up so the
                # matmul stream never starves once it starts (starting
                # earlier on finer pieces just stalls the stream and drops
                # the PE clock out of its ramped state).
                src = xt16[:, j * 512 : (j + 1) * 512]
                pieces = 4 if j == 0 else (2 if j == 1 else 1)
                kk = KC // pieces
                for p in range(pieces):
                    nc.sync.dma_start(
                        xtile[
                            :, p * kk * 512 : (p + 1) * kk * 512
                        ].rearrange("p (k n) -> p k n", n=512),
                        src[p * kk * 128 : (p + 1) * kk * 128, :].rearrange(
                            "(k p) n -> p k n", p=128
                        ),
                    )
                x16t.append(xtile)
            # Scalar ring: w. k0 in three pieces (first matmul only needs the
            # leading 128 columns), then k1..k7 whole.
            w16t = []
            for kc in range(KC):
                wt = wpool.tile([128, D], F16, name=f"w16_{kc}", tag=f"w16_{kc}")
                if kc == 0:
                    nc.scalar.dma_start(wt[:, :512], w16[:, 0, :512])
                    nc.scalar.dma_start(wt[:, 512:], w16[:, 0, 512:])
                else:
                    nc.scalar.dma_start(wt[:], w16[:, kc])
                w16t.append(wt)

            ndma = [0]

            def out_dma(dst, src, j):
                # mid-run outputs ride the Scalar ring (idle after the w
                # loads) so they never contend with x delivery on Sync;
                # late outputs (x long since delivered) alternate rings.
                if j < 6:
                    eng = nc.scalar
                else:
                    eng = nc.sync if ndma[0] % 2 == 0 else nc.scalar
                    ndma[0] += 1
                eng.dma_start(dst, src)

            def mm(ps, mc, kc, j, start, stop):
                nc.tensor.matmul(
                    ps[:],
                    w16t[kc][:, mc * 128 : (mc + 1) * 128],
                    x16t[j][:, kc * 512 : (kc + 1) * 512],
                    start=start,
                    stop=stop,
                )

            # ---- main loop.
            for j in range(JC):
                ps = [
                    ppool.tile([128, 512], F32, name=f"ps{m}", tag=f"ps{m}")
                    for m in range(MC)
                ]
                if j < JC - 1:
                    # k-outer/m-inner; all 8 banks accumulate in parallel.
                    for kc in range(KC):
                        for mc in range(MC):
                            mm(ps[mc], mc, kc, j, kc == 0, kc == KC - 1)
                    # pair drains: DVE even banks / ACT odd banks; one DMA
                    # per pair, alternating rings.
                    for mp in range(MC // 2):
                        osb = opool.tile(
                            [128, 1024], F16, name=f"osb{mp}", tag=f"osb{mp}"
                        )
                        nc.vector.tensor_copy(osb[:, :512], ps[2 * mp][:])
                        nc.scalar.copy(osb[:, 512:], ps[2 * mp + 1][:])
                        out_dma(
                            yt16[:, 2 * mp : 2 * mp + 2, j * 512 : (j + 1) * 512],
                            osb[:].rearrange("p (m n) -> p m n", n=512),
                            j,
                        )
                else:
                    # j7: m-outer/k-inner with eager per-bank drain+store so
                    # the kernel tail is one small DMA, not 8 banks' worth.
                    for mc in range(MC):
                        for kc in range(KC):
                            mm(ps[mc], mc, kc, j, kc == 0, kc == KC - 1)
                        osb = opool.tile(
                            [128, 512], F16, name=f"osl{mc}", tag=f"osl{mc}"
                        )
                        if mc < MC - 1:
                            if mc % 2 == 0:
                                nc.vector.tensor_copy(osb[:], ps[mc][:])
                            else:
                                nc.scalar.copy(osb[:], ps[mc][:])
                            out_dma(yt16[:, mc, j * 512 : (j + 1) * 512], osb[:], j)
                        else:
                            # final bank: both half drains on DVE (keeps the
                            # Scalar queue free to fire its DMA trigger the
                            # moment the copy lands), 64KB DMAs on both rings.
                            nc.vector.tensor_copy(osb[:, :256], ps[mc][:, :256])
                            nc.vector.tensor_copy(osb[:, 256:], ps[mc][:, 256:])
                            base = j * 512
                            nc.scalar.dma_start(
                                yt16[:, mc, base : base + 256], osb[:, :256]
                            )
                            nc.sync.dma_start(
                                yt16[:, mc, base + 256 : base + 512], osb[:, 256:]
                            )

    nc.compile()
    _NC_CACHE = nc
    return nc


def _run(x: np.ndarray, cores: np.ndarray, trace: bool = False, trace_cores=None):
    from concourse.bass_utils import run_bass_kernel_spmd

    W = build_w(cores)
    x16 = x.astype(np.float16)
    w16d = np.ascontiguousarray(
        W.astype(np.float32).astype(np.float16).reshape(KC, 128, D).transpose(1, 0, 2)
    )

    in_maps = []
    for ci in range(N_CORES):
        sl = slice(ci * NPC, (ci + 1) * NPC)
        xt16_c = np.ascontiguousarray(x16[sl].T)
        in_maps.append({"xt16": xt16_c, "w16": w16d})

    nc = _build_bass()
    kwargs = {}
    if trace_cores is not None:
        kwargs["trace_cores"] = trace_cores
    res = run_bass_kernel_spmd(
        nc, in_maps, core_ids=list(range(N_CORES)), trace=trace, **kwargs
    )

    y = np.empty((B, D), dtype=np.float32)
    for ci in range(N_CORES):
        # yt16 [128, MC, NPC] -> [NPC, D]
        y[ci * NPC : (ci + 1) * NPC, :] = (
            res.results[ci]["yt16"].astype(np.float32).transpose(2, 1, 0).reshape(NPC, D)
        )
    return y, res


def kernel(x: np.ndarray, cores: np.ndarray) -> np.ndarray:
    y, _ = _run(x, cores, trace=False)
    return y
